# revision 20
# baseline (speedup 1.0000x reference)
"""Trainium2 Bass kernel for nn_ACLFTransformer (dual-encoder transformer).

Sharding: pure data-parallel — batch element i runs entirely on core i
(B=8 == n_cores=8), weights replicated per core, no collectives.

Per-core single-batch forward:
  - residual stream kept in fp32, normal layout [l(part), d(free)], 4 tiles
  - all matmuls in bf16 (fp32 PSUM accumulation), weights host-pretransposed
    to [in, out]
  - attention computed as scores^T [k, q]: key-mask folds into the exp's
    per-partition bias, denominator comes from a ones-column appended to V's
    stationary operand, per-head 1/den applied in a small transpose chain
  - decoder self-attention is block-sparse causal (skips fully-masked k/q
    blocks, one DVE bias add on diagonal blocks)
  - generator streams the 32000-vocab projection, keeps logits in SBUF
    (bf16), accumulates sum(exp) via the activation accum_out port, then
    subtracts logsumexp in a second SBUF pass
"""
import sys
import types

import numpy as np

P = 128
D = 512
H = 8
DKH = 64
FF = 2048
VOCAB = 32000
L = 512
NC = 4  # D // P chunks
SQD = float(np.sqrt(D))

_COMPILED = {}
LAST_EXEC_NS = [None]
LAST_RES = [None]
TRACE = [False]


# ----------------------------------------------------------------- runtime --
def _install_ntff_hook():
    try:
        import antenv.axon_hooks  # noqa
        return
    except ImportError:
        pass
    try:
        import antenv
        from trn_agent_boot.trn_boot import _ntff_profile_via_ctypes
    except ImportError:
        return
    mod = types.ModuleType("antenv.axon_hooks")
    _hook = [None]
    mod.set_axon_ntff_profile_hook = lambda h: _hook.__setitem__(0, h)
    mod.get_axon_ntff_profile_hook = lambda: _hook[0]
    sys.modules["antenv.axon_hooks"] = mod
    antenv.axon_hooks = mod
    mod.set_axon_ntff_profile_hook(_ntff_profile_via_ctypes('/opt/axon/libaxon_pjrt.so'))


def _run_spmd(nc, in_maps, trace):
    import concourse.bass_utils as bu
    if trace:
        _install_ntff_hook()
        bu.upload_artifacts = lambda tmpdir: "local://skipped"
    return bu.run_bass_kernel_spmd(
        nc, in_maps, core_ids=list(range(len(in_maps))), trace=trace)


# ----------------------------------------------------------------- builder --
class Builder:
    def __init__(self, nc, tc, cfg):
        import concourse.mybir as mybir
        import concourse.bass as bass
        self.bass = bass
        self.mybir = mybir
        self.AF = mybir.ActivationFunctionType
        self.OP = mybir.AluOpType
        self.F32 = mybir.dt.float32
        self.BF16 = mybir.dt.bfloat16
        self.I32 = mybir.dt.int32
        self.AX = mybir.AxisListType
        self.nc = nc
        self.tc = tc
        self.cfg = cfg
        self.inputs = {}

    def dram_in(self, name, shape, dtype):
        h = self.nc.dram_tensor(name, shape, dtype, kind="ExternalInput")
        self.inputs[name] = h
        return h

    def eps_tile(self, val):
        if not hasattr(self, "_eps_tiles"):
            self._eps_tiles = {}
        if val not in self._eps_tiles:
            t = self.const_pool.tile([P, 1], self.F32,
                                     tag=f"eps{len(self._eps_tiles)}", bufs=1)
            self.nc.vector.memset(t[:], float(val))
            self._eps_tiles[val] = t
        return self._eps_tiles[val][:]

    # ---- building blocks ----------------------------------------------
    def embed(self, xpool, tag, tok_dram, emb_dram, pe_tiles, work):
        nc = self.nc
        X = []
        for t in range(NC):
            idx = work.tile([P, 1], self.I32, tag="idx", bufs=2)
            nc.sync.dma_start(idx[:], tok_dram[t * P:(t + 1) * P, :])
            g = work.tile([P, D], self.BF16, tag="gather", bufs=2)
            nc.gpsimd.indirect_dma_start(
                out=g[:], out_offset=None, in_=emb_dram[:],
                in_offset=self.bass.IndirectOffsetOnAxis(ap=idx[:, :1], axis=0))
            x = xpool.tile([P, D], self.F32, tag=f"{tag}{t}", bufs=1)
            nc.vector.tensor_add(x[:], g[:], pe_tiles[t][:])
            X.append(x)
        return X

    def layernorm(self, X, work, out_dtype, unbiased, eps_in, out_pool=None,
                  out_tag="y", out_bufs=1):
        nc, AF, OP = self.nc, self.AF, self.OP
        denom = 1.0 / (D - 1) if unbiased else 1.0 / D
        Y = []
        pool = out_pool or work
        for t in range(NC):
            s = work.tile([P, 1], self.F32, tag="ln_s", bufs=2)
            nc.vector.reduce_sum(s[:], X[t][:], axis=self.AX.X)
            mean = work.tile([P, 1], self.F32, tag="ln_m", bufs=2)
            nc.vector.tensor_scalar_mul(mean[:], s[:], 1.0 / D)
            trash = work.tile([P, D], self.BF16, tag="ln_tr", bufs=2)
            ssq = work.tile([P, 1], self.F32, tag="ln_ss", bufs=2)
            nc.vector.scalar_tensor_tensor(
                out=trash[:], in0=X[t][:], scalar=mean[:], in1=X[t][:],
                op0=OP.subtract, op1=OP.mult, accum_out=ssq[:])
            lnv = work.tile([P, 1], self.F32, tag="ln_lv", bufs=2)
            nc.scalar.activation(lnv[:], ssq[:], AF.Ln, scale=denom,
                                 bias=self.eps_tile(eps_in) if eps_in else 0.0)
            rstd = work.tile([P, 1], self.F32, tag="ln_rs", bufs=2)
            nc.scalar.activation(rstd[:], lnv[:], AF.Exp, scale=-0.5)
            negmr = work.tile([P, 1], self.F32, tag="ln_nm", bufs=2)
            nc.vector.tensor_scalar(
                out=negmr[:], in0=mean[:], scalar1=rstd[:], scalar2=-1.0,
                op0=OP.mult, op1=OP.mult)
            y = pool.tile([P, D], out_dtype, tag=f"{out_tag}{t}", bufs=out_bufs)
            nc.scalar.activation(y[:], X[t][:], AF.Identity,
                                 bias=negmr[:], scale=rstd[:])
            Y.append(y)
        return Y

    def transpose_to(self, Y, work, psum, out_tag="yT", out_pool=None):
        nc = self.nc
        pool = out_pool or work
        YT = []
        for c in range(NC):
            pt = psum.tile([P, D], self.BF16, tag="tp", bufs=1)
            for t in range(NC):
                nc.tensor.transpose(pt[:, t * P:(t + 1) * P],
                                    Y[t][:, c * P:(c + 1) * P], self.idn[:])
            yt = pool.tile([P, D], self.BF16, tag=f"{out_tag}{c}", bufs=2)
            nc.scalar.activation(yt[:], pt[:], self.AF.Copy)
            YT.append(yt)
        return YT

    def load_w(self, name, wpool, tag, cols=D):
        nc = self.nc
        wd = self.inputs[name]
        tiles = []
        for c in range(NC):
            w = wpool.tile([P, cols], self.BF16, tag=f"{tag}{c}", bufs=1)
            nc.sync.dma_start(w[:], wd[c * P:(c + 1) * P, :])
            tiles.append(w)
        return tiles

    def linear_T(self, W, XT, work, psum, out_tag):
        nc = self.nc
        out = []
        for oc in range(NC):
            pt = psum.tile([P, D], self.F32, tag="lin", bufs=2)
            for c in range(NC):
                nc.tensor.matmul(pt[:], lhsT=W[c][:, oc * P:(oc + 1) * P],
                                 rhs=XT[c][:], start=(c == 0), stop=(c == NC - 1))
            o = work.tile([P, D], self.BF16, tag=f"{out_tag}{oc}", bufs=2)
            nc.scalar.activation(o[:], pt[:], self.AF.Copy)
            out.append(o)
        return out

    def linear_V(self, W, XT, work, psum):
        nc = self.nc
        out = []
        for t in range(NC):
            pt = psum.tile([P, D], self.F32, tag="lin", bufs=2)
            for c in range(NC):
                nc.tensor.matmul(pt[:], lhsT=XT[c][:, t * P:(t + 1) * P],
                                 rhs=W[c][:], start=(c == 0), stop=(c == NC - 1))
            v = work.tile([P, H * (DKH + 1)], self.BF16, tag=f"v{t}", bufs=2)
            src3 = pt[:].rearrange("p (h d) -> p h d", h=H)
            dst3 = v[:].rearrange("p (h d) -> p h d", h=H, d=DKH + 1)[:, :, 0:DKH]
            nc.vector.tensor_copy(dst3, src3)
            nc.vector.memset(
                v[:].rearrange("p (h d) -> p h d", h=H, d=DKH + 1)[:, :, DKH:DKH + 1],
                1.0)
            out.append(v)
        return out

    def _attention(self, QT, KT, Vt, work, psum, psum2, mask_mode, kbias,
                   tbias_tiles):
        nc, AF = self.nc, self.AF
        avT = [work.tile([P, D], self.BF16, tag=f"avT{c}", bufs=2,
                         name=f"avT{c}") for c in range(NC)]
        for h in range(H):
            ht, hp = h // 2, (h % 2) * DKH
            q_ap = QT[ht][hp:hp + DKH, :]
            avp = psum2.tile([DKH + 1, D], self.F32, tag="avp", bufs=1)
            for kc in range(NC):
                q0 = kc * P if mask_mode == "causal" else 0
                n = D - q0
                sp = psum.tile([P, D], self.F32, tag="sp", bufs=3)
                nc.tensor.matmul(sp[:, 0:n],
                                 lhsT=KT[ht][hp:hp + DKH, kc * P:(kc + 1) * P],
                                 rhs=q_ap[:, q0:D], start=True, stop=True)
                if mask_mode == "causal":
                    nc.vector.tensor_add(sp[:, 0:P], sp[:, 0:P],
                                         self.causal_diag[:])
                elif mask_mode == "full":
                    nc.vector.tensor_add(sp[:, 0:n], sp[:, 0:n],
                                         tbias_tiles[kc][:, q0:D])
                et = work.tile([P, D], self.BF16, tag="et", bufs=4)
                if mask_mode == "kbias":
                    nc.scalar.activation(et[:, 0:n], sp[:, 0:n], AF.Exp,
                                         bias=kbias[:, kc:kc + 1], scale=0.125)
                else:
                    nc.scalar.activation(et[:, 0:n], sp[:, 0:n], AF.Exp,
                                         scale=0.125)
                nc.tensor.matmul(avp[:, q0:D],
                                 lhsT=Vt[kc][:, h * (DKH + 1):(h + 1) * (DKH + 1)],
                                 rhs=et[:, 0:n], start=(kc == 0),
                                 stop=(kc == NC - 1))
            # avp rows 0..63 hold av^T (already the layout O-proj needs);
            # row 64 holds the softmax denominators. Broadcast 1/den to all
            # 64 partitions via a K=1 matmul, then one multiply finishes it.
            av_sb = work.tile([DKH + 1, D], self.F32, tag="av_sb", bufs=2)
            nc.scalar.activation(av_sb[:], avp[:], AF.Copy)
            F32R = self.mybir.dt.float32r
            rcp_sb = work.tile([1, D], F32R, tag="rden", bufs=2)
            with nc.allow_low_precision(reason="f32r recip feeds bcast mm"):
                nc.vector.reciprocal(rcp_sb[:], av_sb[DKH:DKH + 1, :])
            r64 = psum2.tile([DKH, D], self.F32, tag="avTp", bufs=1)
            nc.tensor.matmul(r64[:], lhsT=self.ones64[:].bitcast(F32R),
                             rhs=rcp_sb[:], start=True, stop=True)
            nc.vector.tensor_tensor(out=avT[ht][hp:hp + DKH, :],
                                    in0=av_sb[0:DKH, :], in1=r64[:],
                                    op=self.OP.mult)
        return avT

    def oproj_resid(self, avT, Wo, X, psum, acc=None, res_scale=None,
                    acc_add=False):
        nc = self.nc
        for t in range(NC):
            pt = psum.tile([P, D], self.F32, tag="lin", bufs=2)
            for c in range(NC):
                nc.tensor.matmul(pt[:], lhsT=avT[c][:, t * P:(t + 1) * P],
                                 rhs=Wo[c][:], start=(c == 0), stop=(c == NC - 1))
            if acc is not None and not acc_add:
                nc.vector.scalar_tensor_tensor(
                    out=acc[t][:], in0=X[t][:], scalar=float(res_scale),
                    in1=pt[:], op0=self.OP.mult, op1=self.OP.add)
            elif acc is not None:
                nc.vector.tensor_add(acc[t][:], acc[t][:], pt[:])
            else:
                nc.vector.tensor_add(X[t][:], X[t][:], pt[:])

    def kv_proj(self, src_T, prefix, work, wpool, psum):
        WK = self.load_w(prefix + "kw", wpool, "wk")
        KT = self.linear_T(WK, src_T, work, psum, "kT")
        WV = self.load_w(prefix + "vw", wpool, "wv")
        Vt = self.linear_V(WV, src_T, work, psum)
        return KT, Vt

    def mha(self, YT_q, src_T, X, prefix, mask_mode, work, wpool, psum, psum2,
            kbias=None, tbias_tiles=None, acc=None, res_scale=None,
            acc_add=False, kv=None):
        """src_T: transposed source for K/V (y for self, mem for cross)."""
        if kv is None:
            kv = self.kv_proj(src_T, prefix, work, wpool, psum)
        KT, Vt = kv
        WQ = self.load_w(prefix + "qw", wpool, "wq")
        QT = self.linear_T(WQ, YT_q, work, psum, "qT")
        avT = self._attention(QT, KT, Vt, work, psum, psum2, mask_mode,
                              kbias, tbias_tiles)
        WO = self.load_w(prefix + "ow", wpool, "wo")
        self.oproj_resid(avT, WO, X, psum, acc=acc, res_scale=res_scale,
                         acc_add=acc_add)

    def ffn(self, X, prefix, work, wpool, psum):
        nc, AF = self.nc, self.AF
        Y = self.layernorm(X, work, self.BF16, unbiased=True, eps_in=0.0)
        YT = self.transpose_to(Y, work, psum)
        W1 = []
        for c in range(NC):
            w = wpool.tile([P, FF], self.BF16, tag=f"w1_{c}", bufs=1)
            nc.sync.dma_start(w[:], self.inputs[prefix + "w1"][c * P:(c + 1) * P, :])
            W1.append(w)
        H1 = []
        for oc in range(FF // P):
            pt = psum.tile([P, D], self.F32, tag="lin", bufs=2)
            for c in range(NC):
                nc.tensor.matmul(pt[:], lhsT=W1[c][:, oc * P:(oc + 1) * P],
                                 rhs=YT[c][:], start=(c == 0), stop=(c == NC - 1))
            h1 = work.tile([P, D], self.BF16, tag=f"h1_{oc}", bufs=1)
            nc.scalar.activation(h1[:], pt[:], AF.Relu)
            H1.append(h1)
        W2 = []
        for c in range(FF // P):
            w = wpool.tile([P, D], self.BF16, tag=f"w2_{c}", bufs=1)
            nc.sync.dma_start(w[:], self.inputs[prefix + "w2"][c * P:(c + 1) * P, :])
            W2.append(w)
        for t in range(NC):
            pt = psum.tile([P, D], self.F32, tag="lin", bufs=2)
            for c in range(FF // P):
                nc.tensor.matmul(pt[:], lhsT=H1[c][:, t * P:(t + 1) * P],
                                 rhs=W2[c][:], start=(c == 0),
                                 stop=(c == FF // P - 1))
            nc.vector.tensor_add(X[t][:], X[t][:], pt[:])

    def enc_layer(self, X, prefix, kbias, work, wpool, psum, psum2):
        Y = self.layernorm(X, work, self.BF16, unbiased=True, eps_in=0.0)
        YT = self.transpose_to(Y, work, psum)
        self.mha(YT, YT, X, prefix, "kbias", work, wpool, psum, psum2,
                 kbias=kbias)
        self.ffn(X, prefix, work, wpool, psum)

    def dec_layer(self, X, prefix, mem1T, mem2T, kb1, kb2, tbias, work,
                  wpool, psum, psum2, acc, xpool, xtag, kv1, next_prefix):
        Y = self.layernorm(X, work, self.BF16, unbiased=True, eps_in=0.0)
        YT = self.transpose_to(Y, work, psum)
        self.mha(YT, YT, X, prefix + "self_", self.cfg["tgt_mode"], work,
                 wpool, psum, psum2, tbias_tiles=tbias)
        kv2 = self.kv_proj(mem2T, prefix + "src2_", work, wpool, psum)
        Y1 = self.layernorm(X, work, self.BF16, unbiased=True, eps_in=0.0)
        YT1 = self.transpose_to(Y1, work, psum)
        self.mha(YT1, mem1T, X, prefix + "src1_", "kbias", work, wpool, psum,
                 psum2, kbias=kb1, acc=acc, res_scale=2.0, kv=kv1)
        Y2 = self.layernorm(X, work, self.BF16, unbiased=True, eps_in=0.0)
        YT2 = self.transpose_to(Y2, work, psum)
        self.mha(YT2, mem2T, X, prefix + "src2_", "kbias", work, wpool, psum,
                 psum2, kbias=kb2, acc=acc, acc_add=True, kv=kv2)
        Xn = self.layernorm(acc, work, self.F32, unbiased=False, eps_in=1e-5,
                            out_pool=xpool, out_tag=xtag)
        X[:] = Xn
        kv1_next = None
        if next_prefix is not None:
            # next layer's cross1 K/V: independent PE work that covers this
            # FFN's layernorm latency
            kv1_next = self.kv_proj(mem1T, next_prefix + "src1_", work,
                                    wpool, psum)
        self.ffn(X, prefix, work, wpool, psum)
        return kv1_next

    def generator(self, YT, lgpool, gwpool, gwork, psum):
        nc, AF = self.nc, self.AF
        out_dram = self.out
        vchunks = []
        off = 0
        while off < VOCAB:
            n = min(512, VOCAB - off)
            vchunks.append((off, n))
            off += n
        nvc = len(vchunks)
        VB = 4  # vchunks per block

        p2_count = [0]

        def pass2_chunk(prev, off, n):
            plg, pnlse, pt = prev
            ob = gwork.tile([P, 512], self.F32, tag="ob", bufs=4, name="ob")
            if p2_count[0] % 2 == 0:
                nc.vector.tensor_scalar_add(ob[:, 0:n], plg[:, off:off + n],
                                            pnlse[:])
            else:
                nc.scalar.activation(ob[:, 0:n], plg[:, off:off + n],
                                     AF.Identity, bias=pnlse[:])
            p2_count[0] += 1
            nc.sync.dma_start(out_dram[pt * P:(pt + 1) * P, off:off + n],
                              ob[:, 0:n])

        prev = None
        for t in range(NC):
            lg = lgpool.tile([P, VOCAB], self.BF16, tag="lg", bufs=2)
            se = gwork.tile([P, nvc], self.F32, tag="se", bufs=2)
            p2 = iter(vchunks if prev else ())
            for b0 in range(0, nvc, VB):
                block = vchunks[b0:b0 + VB]
                span0 = block[0][0]
                span_n = block[-1][0] + block[-1][1] - span0
                gw = []
                for c in range(NC):
                    w = gwpool.tile([P, 512 * VB], self.BF16, tag=f"gw{c}",
                                    bufs=2, name=f"gw{c}")
                    nc.sync.dma_start(
                        w[:, 0:span_n],
                        self.inputs["genw"][c * P:(c + 1) * P,
                                            span0:span0 + span_n])
                    gw.append(w)
                pts = [psum.tile([P, 512], self.F32, tag="glin", bufs=VB,
                                 name=f"glin{bi}") for bi in range(len(block))]
                for c in range(NC):
                    for bi, (off, n) in enumerate(block):
                        nc.tensor.matmul(pts[bi][:, 0:n],
                                         lhsT=YT[c][:, t * P:(t + 1) * P],
                                         rhs=gw[c][:, bi * 512:bi * 512 + n],
                                         start=(c == 0), stop=(c == NC - 1))
                for bi, (off, n) in enumerate(block):
                    trash = gwork.tile([P, 512], self.BF16, tag="gtrash",
                                       bufs=4)
                    nc.scalar.activation(trash[:, 0:n], pts[bi][:, 0:n],
                                         AF.Exp,
                                         accum_out=se[:, b0 + bi:b0 + bi + 1])
                    nc.vector.tensor_copy(lg[:, off:off + n], pts[bi][:, 0:n])
                    nxt = next(p2, None)
                    if nxt is not None:
                        pass2_chunk(prev, *nxt)
            for nxt in p2:
                pass2_chunk(prev, *nxt)
            ssum = gwork.tile([P, 1], self.F32, tag="ssum", bufs=2)
            nc.vector.reduce_sum(ssum[:], se[:], axis=self.AX.X)
            lse = gwork.tile([P, 1], self.F32, tag="lse", bufs=2)
            nc.scalar.activation(lse[:], ssum[:], AF.Ln)
            nlse = gwork.tile([P, 1], self.F32, tag="nlse", bufs=2)
            nc.vector.tensor_scalar_mul(nlse[:], lse[:], -1.0)
            prev = (lg, nlse, t)
        for off, n in vchunks:
            pass2_chunk(prev, off, n)


LDW_OPT = [False]


def _patch_toolchain():
    """Restrict ACT func-table sets to one covering set so the table is
    loaded once instead of being swapped between funcs (~1.5us per swap)."""
    import concourse.hw_specs as hw_specs
    import concourse.bacc as bacc_mod
    import concourse.bass_utils as bu
    from concourse import mybir
    AF = mybir.ActivationFunctionType
    needed = {AF.Exp, AF.Ln, AF.Copy, AF.Identity, AF.Relu}
    if getattr(bacc_mod, "_act_tables_patched", False) is False:
        orig = hw_specs.get_activation_tables

        def patched(arch):
            t = orig(arch)
            cover = [k for k, v in t.items() if needed <= v]
            if not cover:
                return t
            pick = cover[0]
            return {k: (v if k == pick else set()) for k, v in t.items()}

        bacc_mod.get_activation_tables = patched
        bacc_mod._act_tables_patched = True
    if LDW_OPT[0] and getattr(bu, "_ldw_patched", False) is False:
        orig_run = bu.run_command

        def run2(cmd, **kw):
            cmd = [c.replace("--enable-ldw-opt=false", "--enable-ldw-opt=true")
                   if isinstance(c, str) else c for c in cmd]
            return orig_run(cmd, **kw)

        bu.run_command = run2
        bu._ldw_patched = True


def build_model(cfg):
    import concourse.mybir as mybir
    import concourse.tile as tile
    from concourse import bacc
    from concourse.masks import make_identity

    _patch_toolchain()
    nc = bacc.Bacc("TRN2", target_bir_lowering=False, debug=False)
    with tile.TileContext(nc) as tc:
        b = Builder(nc, tc, cfg)
        F32, BF16, I32 = b.F32, b.BF16, b.I32
        for nm in ("src", "src2", "tgt"):
            b.dram_in(nm, [L, 1], I32)
        b.dram_in("pe", [L, D], F32)
        b.dram_in("kbias1", [P, NC], F32)
        b.dram_in("kbias2", [P, NC], F32)
        if cfg["tgt_mode"] == "causal":
            b.dram_in("causal_diag", [P, P], F32)
        elif cfg["tgt_mode"] == "full":
            b.dram_in("tbias", [L, L], F32)
        for e in (1, 2, 3):
            b.dram_in(f"emb{e}", [VOCAB, D], BF16)
        for enc in (1, 2):
            for j in range(cfg["n_enc"]):
                pfx = f"e{enc}{j}_"
                for w in ("qw", "kw", "vw", "ow"):
                    b.dram_in(pfx + w, [D, D], BF16)
                b.dram_in(pfx + "w1", [D, FF], BF16)
                b.dram_in(pfx + "w2", [FF, D], BF16)
        for j in range(cfg["n_dec"]):
            pfx = f"d{j}_"
            for att in ("self_", "src1_", "src2_"):
                for w in ("qw", "kw", "vw", "ow"):
                    b.dram_in(pfx + att + w, [D, D], BF16)
            b.dram_in(pfx + "w1", [D, FF], BF16)
            b.dram_in(pfx + "w2", [FF, D], BF16)
        if cfg["gen"]:
            b.dram_in("genw", [D, VOCAB], BF16)
        b.out = nc.dram_tensor("out", [L, VOCAB], mybir.dt.float32,
                               kind="ExternalOutput")
        dbg = cfg.get("dbg")
        if dbg:
            b.dbg = nc.dram_tensor("dbg", [L, D], mybir.dt.float32,
                                   kind="ExternalOutput")

        with tc.tile_pool(name="const", bufs=1) as const, \
             tc.tile_pool(name="xfer", bufs=1) as xfer:

            def dump_dbg(tiles):
                for t in range(NC):
                    ot = const.tile([P, D], F32, tag=f"dbgo{t}", bufs=1)
                    nc.scalar.activation(ot[:], tiles[t][:], b.AF.Copy)
                    nc.sync.dma_start(b.dbg[t * P:(t + 1) * P, :], ot[:])

            b.const_pool = const
            b.idn = const.tile([P, P], BF16, tag="idn", bufs=1)
            make_identity(nc, b.idn[:])
            b.ones64 = const.tile([1, DKH], F32, tag="ones64", bufs=1)
            nc.vector.memset(b.ones64[:], 1.0)
            kb1 = const.tile([P, NC], F32, tag="kb1", bufs=1)
            nc.sync.dma_start(kb1[:], b.inputs["kbias1"][:])
            kb2 = const.tile([P, NC], F32, tag="kb2", bufs=1)
            nc.sync.dma_start(kb2[:], b.inputs["kbias2"][:])
            tbias = None
            if cfg["tgt_mode"] == "causal":
                cd = const.tile([P, P], F32, tag="cd", bufs=1)
                nc.sync.dma_start(cd[:], b.inputs["causal_diag"][:])
                b.causal_diag = cd
            elif cfg["tgt_mode"] == "full":
                tbias = []
                for kc in range(NC):
                    tb = const.tile([P, L], F32, tag=f"tb{kc}", bufs=1)
                    nc.sync.dma_start(tb[:], b.inputs["tbias"][kc * P:(kc + 1) * P, :])
                    tbias.append(tb)

            with tc.tile_pool(name="encdec", bufs=2) as work, \
                 tc.tile_pool(name="wpool", bufs=1) as wpool, \
                 tc.tile_pool(name="xpool", bufs=1) as xpool, \
                 tc.tile_pool(name="pepool", bufs=1) as pepool, \
                 tc.tile_pool(name="psum", bufs=1, space="PSUM") as psum, \
                 tc.tile_pool(name="psum2", bufs=1, space="PSUM") as psum2:
                pe_tiles = []
                for t in range(NC):
                    pt_ = pepool.tile([P, D], F32, tag=f"pe{t}", bufs=1)
                    nc.sync.dma_start(pt_[:], b.inputs["pe"][t * P:(t + 1) * P, :])
                    pe_tiles.append(pt_)

                X1 = b.embed(xpool, "x1_", b.inputs["src"], b.inputs["emb1"],
                             pe_tiles, work)
                X2 = b.embed(xpool, "x2_", b.inputs["src2"], b.inputs["emb2"],
                             pe_tiles, work)
                if dbg == "emb1":
                    dump_dbg(X1)
                if dbg == "emb2":
                    dump_dbg(X2)
                # interleave the two (independent) encoders so one's matmuls
                # cover the other's layernorm latency
                for j in range(cfg["n_enc"]):
                    b.enc_layer(X1, f"e1{j}_", kb1, work, wpool, psum, psum2)
                    b.enc_layer(X2, f"e2{j}_", kb2, work, wpool, psum, psum2)
                if dbg == "enc1":
                    dump_dbg(b.layernorm(X1, work, F32, unbiased=True,
                                         eps_in=0.0, out_tag="yfd"))
                if dbg == "enc2":
                    dump_dbg(b.layernorm(X2, work, F32, unbiased=True,
                                         eps_in=0.0, out_tag="yfd"))
                Yf1 = b.layernorm(X1, work, BF16, unbiased=True, eps_in=0.0)
                mem1T = b.transpose_to(Yf1, work, psum, out_tag="m1_",
                                       out_pool=xfer)
                Yf2 = b.layernorm(X2, work, BF16, unbiased=True, eps_in=0.0)
                mem2T = b.transpose_to(Yf2, work, psum, out_tag="m2_",
                                       out_pool=xfer)

                Xd = b.embed(xpool, "x1_", b.inputs["tgt"], b.inputs["emb3"],
                             pe_tiles, work)
                if dbg == "emb3":
                    dump_dbg(Xd)
                acc = [xpool.tile([P, D], F32, tag=f"acc{t}", bufs=1,
                                  name=f"acc{t}") for t in range(NC)]
                kv1 = b.kv_proj(mem1T, "d0_src1_", work, wpool, psum)
                for j in range(cfg["n_dec"]):
                    nxt = f"d{j + 1}_" if j + 1 < cfg["n_dec"] else None
                    kv1 = b.dec_layer(Xd, f"d{j}_", mem1T, mem2T, kb1, kb2,
                                      tbias, work, wpool, psum, psum2, acc,
                                      xpool, "x1_", kv1, nxt)
                if dbg == "dec":
                    Yd32 = b.layernorm(Xd, work, F32, unbiased=True,
                                       eps_in=0.0, out_tag="ydd")
                    dump_dbg(Yd32)
                Yg = b.layernorm(Xd, work, BF16, unbiased=True, eps_in=0.0)
                YgT = b.transpose_to(Yg, work, psum, out_tag="ygT",
                                     out_pool=xfer)

            if cfg["gen"]:
                with tc.tile_pool(name="lg", bufs=1) as lgpool, \
                     tc.tile_pool(name="gw", bufs=1) as gwpool, \
                     tc.tile_pool(name="gwork", bufs=1) as gwork, \
                     tc.tile_pool(name="psumg", bufs=1, space="PSUM") as psumg:
                    b.generator(YgT, lgpool, gwpool, gwork, psumg)
    nc.compile()
    return nc


# -------------------------------------------------------------------- host --
def _pos_encoding(max_len, d):
    pos = np.arange(max_len, dtype=np.float32)[:, None]
    div = np.exp(np.arange(0, d, 2, dtype=np.float32) * (-np.log(10000.0) / d))
    pe = np.zeros((max_len, d), np.float32)
    pe[:, 0::2] = np.sin(pos * div)
    pe[:, 1::2] = np.cos(pos * div)
    return pe


def _prep_weights(params):
    import ml_dtypes
    bf16 = ml_dtypes.bfloat16
    out = {}
    nonzero_bias = False
    ln_affine = False

    def lin(dst, p):
        nonlocal nonzero_bias
        w = np.asarray(p["w"], np.float32)
        out[dst] = np.ascontiguousarray(w.T).astype(bf16)
        if np.any(np.asarray(p["b"]) != 0):
            nonzero_bias = True

    def ln_check(p):
        nonlocal ln_affine
        if np.any(np.asarray(p["a"]) != 1) or np.any(np.asarray(p["b"]) != 0):
            ln_affine = True

    for e in (1, 2, 3):
        emb = np.asarray(params[f"emb{e}"], np.float32) * SQD
        out[f"emb{e}"] = emb.astype(bf16)
    n_enc = len(params["enc1"]["layers"])
    for enc in (1, 2):
        pp = params[f"enc{enc}"]
        for j, lp in enumerate(pp["layers"]):
            pfx = f"e{enc}{j}_"
            for k, nm in (("q", "qw"), ("k", "kw"), ("v", "vw"), ("o", "ow")):
                lin(pfx + nm, lp["attn"][k])
            lin(pfx + "w1", lp["ff"]["w1"])
            lin(pfx + "w2", lp["ff"]["w2"])
            ln_check(lp["n1"]); ln_check(lp["n2"])
        ln_check(pp["norm"])
    n_dec = len(params["dec"]["layers"])
    for j, lp in enumerate(params["dec"]["layers"]):
        pfx = f"d{j}_"
        for att, key in (("self_", "self"), ("src1_", "src1"), ("src2_", "src2")):
            for k, nm in (("q", "qw"), ("k", "kw"), ("v", "vw"), ("o", "ow")):
                lin(pfx + att + nm, lp[key][k])
        lin(pfx + "w1", lp["ff"]["w1"])
        lin(pfx + "w2", lp["ff"]["w2"])
        for n in lp["n"]:
            ln_check(n)
        ln_check(lp["lay"])
    ln_check(params["dec"]["norm"])
    lin("genw", params["gen"])
    return out, n_enc, n_dec, nonzero_bias, ln_affine


def kernel(src, src2, tgt, src_mask, src2_mask, tgt_mask, params):
    src = np.asarray(src); src2 = np.asarray(src2); tgt = np.asarray(tgt)
    src_mask = np.asarray(src_mask); src2_mask = np.asarray(src2_mask)
    tgt_mask = np.asarray(tgt_mask)
    B = src.shape[0]

    weights, n_enc, n_dec, nonzero_bias, ln_affine = _prep_weights(params)
    if nonzero_bias or ln_affine:
        raise NotImplementedError(
            "nonzero linear bias / affine LN not supported by this kernel")

    tril = np.tril(np.ones((L, L), np.int32))
    causal = all(np.array_equal(np.asarray(tgt_mask[i]).astype(np.int32), tril)
                 for i in range(B))
    tgt_any_mask = bool(np.any(np.asarray(tgt_mask) == 0))
    tgt_mode = "causal" if causal else ("full" if tgt_any_mask else "none")

    cfg_key = (n_enc, n_dec, tgt_mode)
    if cfg_key not in _COMPILED:
        cfg = {"n_enc": n_enc, "n_dec": n_dec, "tgt_mode": tgt_mode,
               "gen": True, "dbg": None}
        _COMPILED[cfg_key] = build_model(cfg)
    nc = _COMPILED[cfg_key]

    pe = _pos_encoding(L, D)
    q = np.arange(P, dtype=np.float32)
    causal_diag = np.where(q[:, None] > q[None, :], -8e9, 0.0).astype(np.float32)

    in_maps = []
    for i in range(B):
        m = dict(weights)
        m["src"] = np.ascontiguousarray(src[i].reshape(L, 1)).astype(np.int32)
        m["src2"] = np.ascontiguousarray(src2[i].reshape(L, 1)).astype(np.int32)
        m["tgt"] = np.ascontiguousarray(tgt[i].reshape(L, 1)).astype(np.int32)
        m["pe"] = pe
        m["kbias1"] = np.ascontiguousarray(
            ((src_mask[i, 0].astype(np.float32) - 1) * 1e9).reshape(NC, P).T)
        m["kbias2"] = np.ascontiguousarray(
            ((src2_mask[i, 0].astype(np.float32) - 1) * 1e9).reshape(NC, P).T)
        if tgt_mode == "causal":
            m["causal_diag"] = causal_diag
        elif tgt_mode == "full":
            m["tbias"] = np.ascontiguousarray(
                (tgt_mask[i].astype(np.float32).T - 1) * 8e9)
        in_maps.append(m)

    res = _run_spmd(nc, in_maps, trace=TRACE[0])
    LAST_EXEC_NS[0] = res.exec_time_ns
    LAST_RES[0] = res
    return np.stack([res.results[i]["out"] for i in range(B)]).astype(np.float32)


# revision 22
# speedup vs baseline: 1.1114x; 1.1114x over previous
"""Trainium2 Bass kernel for nn_ACLFTransformer (dual-encoder transformer).

Sharding: pure data-parallel — batch element i runs entirely on core i
(B=8 == n_cores=8), weights replicated per core, no collectives.

Per-core single-batch forward:
  - residual stream kept in fp32, normal layout [l(part), d(free)], 4 tiles
  - all matmuls in bf16 (fp32 PSUM accumulation), weights host-pretransposed
    to [in, out]
  - attention computed as scores^T [k, q]: key-mask folds into the exp's
    per-partition bias, denominator comes from a ones-column appended to V's
    stationary operand, per-head 1/den applied in a small transpose chain
  - decoder self-attention is block-sparse causal (skips fully-masked k/q
    blocks, one DVE bias add on diagonal blocks)
  - generator streams the 32000-vocab projection, keeps logits in SBUF
    (bf16), accumulates sum(exp) via the activation accum_out port, then
    subtracts logsumexp in a second SBUF pass
"""
import sys
import types

import numpy as np

P = 128
D = 512
H = 8
DKH = 64
FF = 2048
VOCAB = 32000
L = 512
NC = 4  # D // P chunks
SQD = float(np.sqrt(D))

_COMPILED = {}
LAST_EXEC_NS = [None]
LAST_RES = [None]
TRACE = [False]


# ----------------------------------------------------------------- runtime --
def _install_ntff_hook():
    try:
        import antenv.axon_hooks  # noqa
        return
    except ImportError:
        pass
    try:
        import antenv
        from trn_agent_boot.trn_boot import _ntff_profile_via_ctypes
    except ImportError:
        return
    mod = types.ModuleType("antenv.axon_hooks")
    _hook = [None]
    mod.set_axon_ntff_profile_hook = lambda h: _hook.__setitem__(0, h)
    mod.get_axon_ntff_profile_hook = lambda: _hook[0]
    sys.modules["antenv.axon_hooks"] = mod
    antenv.axon_hooks = mod
    mod.set_axon_ntff_profile_hook(_ntff_profile_via_ctypes('/opt/axon/libaxon_pjrt.so'))


def _run_spmd(nc, in_maps, trace):
    import concourse.bass_utils as bu
    if trace:
        _install_ntff_hook()
        bu.upload_artifacts = lambda tmpdir: "local://skipped"
    return bu.run_bass_kernel_spmd(
        nc, in_maps, core_ids=list(range(len(in_maps))), trace=trace)


# ----------------------------------------------------------------- builder --
class Builder:
    def __init__(self, nc, tc, cfg):
        import concourse.mybir as mybir
        import concourse.bass as bass
        self.bass = bass
        self.mybir = mybir
        self.AF = mybir.ActivationFunctionType
        self.OP = mybir.AluOpType
        self.F32 = mybir.dt.float32
        self.BF16 = mybir.dt.bfloat16
        self.I32 = mybir.dt.int32
        self.AX = mybir.AxisListType
        self.nc = nc
        self.tc = tc
        self.cfg = cfg
        self.inputs = {}

    def dram_in(self, name, shape, dtype):
        h = self.nc.dram_tensor(name, shape, dtype, kind="ExternalInput")
        self.inputs[name] = h
        return h

    def eps_tile(self, val):
        if not hasattr(self, "_eps_tiles"):
            self._eps_tiles = {}
        if val not in self._eps_tiles:
            t = self.const_pool.tile([P, 1], self.F32,
                                     tag=f"eps{len(self._eps_tiles)}", bufs=1)
            self.nc.vector.memset(t[:], float(val))
            self._eps_tiles[val] = t
        return self._eps_tiles[val][:]

    # ---- building blocks ----------------------------------------------
    def embed(self, xpool, tag, tok_dram, emb_dram, pe_tiles, work):
        nc = self.nc
        X = []
        for t in range(NC):
            idx = work.tile([P, 1], self.I32, tag="idx", bufs=2)
            nc.sync.dma_start(idx[:], tok_dram[t * P:(t + 1) * P, :])
            g = work.tile([P, D], self.BF16, tag="gather", bufs=2)
            nc.gpsimd.indirect_dma_start(
                out=g[:], out_offset=None, in_=emb_dram[:],
                in_offset=self.bass.IndirectOffsetOnAxis(ap=idx[:, :1], axis=0))
            x = xpool.tile([P, D], self.F32, tag=f"{tag}{t}", bufs=1)
            nc.vector.tensor_add(x[:], g[:], pe_tiles[t][:])
            X.append(x)
        return X

    def layernorm(self, X, work, out_dtype, unbiased, eps_in, out_pool=None,
                  out_tag="y", out_bufs=1):
        nc, AF, OP = self.nc, self.AF, self.OP
        denom = 1.0 / (D - 1) if unbiased else 1.0 / D
        Y = []
        pool = out_pool or work
        for t in range(NC):
            s = work.tile([P, 1], self.F32, tag="ln_s", bufs=2)
            nc.vector.reduce_sum(s[:], X[t][:], axis=self.AX.X)
            mean = work.tile([P, 1], self.F32, tag="ln_m", bufs=2)
            nc.vector.tensor_scalar_mul(mean[:], s[:], 1.0 / D)
            trash = work.tile([P, D], self.BF16, tag="ln_tr", bufs=2)
            ssq = work.tile([P, 1], self.F32, tag="ln_ss", bufs=2)
            nc.vector.scalar_tensor_tensor(
                out=trash[:], in0=X[t][:], scalar=mean[:], in1=X[t][:],
                op0=OP.subtract, op1=OP.mult, accum_out=ssq[:])
            lnv = work.tile([P, 1], self.F32, tag="ln_lv", bufs=2)
            nc.scalar.activation(lnv[:], ssq[:], AF.Ln, scale=denom,
                                 bias=self.eps_tile(eps_in) if eps_in else 0.0)
            rstd = work.tile([P, 1], self.F32, tag="ln_rs", bufs=2)
            nc.scalar.activation(rstd[:], lnv[:], AF.Exp, scale=-0.5)
            negmr = work.tile([P, 1], self.F32, tag="ln_nm", bufs=2)
            nc.vector.tensor_scalar(
                out=negmr[:], in0=mean[:], scalar1=rstd[:], scalar2=-1.0,
                op0=OP.mult, op1=OP.mult)
            y = pool.tile([P, D], out_dtype, tag=f"{out_tag}{t}", bufs=out_bufs)
            nc.scalar.activation(y[:], X[t][:], AF.Identity,
                                 bias=negmr[:], scale=rstd[:])
            Y.append(y)
        return Y

    def transpose_to(self, Y, work, psum, out_tag="yT", out_pool=None):
        nc = self.nc
        pool = out_pool or work
        YT = []
        for c in range(NC):
            pt = psum.tile([P, D], self.BF16, tag="tp", bufs=1)
            for t in range(NC):
                nc.tensor.transpose(pt[:, t * P:(t + 1) * P],
                                    Y[t][:, c * P:(c + 1) * P], self.idn[:])
            yt = pool.tile([P, D], self.BF16, tag=f"{out_tag}{c}", bufs=2)
            nc.scalar.activation(yt[:], pt[:], self.AF.Copy)
            YT.append(yt)
        return YT

    def load_w(self, name, wpool, tag, cols=D):
        nc = self.nc
        wd = self.inputs[name]
        tiles = []
        for c in range(NC):
            w = wpool.tile([P, cols], self.BF16, tag=f"{tag}{c}", bufs=1)
            nc.sync.dma_start(w[:], wd[c * P:(c + 1) * P, :])
            tiles.append(w)
        return tiles

    def linear_T(self, W, XT, work, psum, out_tag):
        nc = self.nc
        out = []
        for oc in range(NC):
            pt = psum.tile([P, D], self.F32, tag="lin", bufs=2)
            for c in range(NC):
                nc.tensor.matmul(pt[:], lhsT=W[c][:, oc * P:(oc + 1) * P],
                                 rhs=XT[c][:], start=(c == 0), stop=(c == NC - 1))
            o = work.tile([P, D], self.BF16, tag=f"{out_tag}{oc}", bufs=2)
            nc.scalar.activation(o[:], pt[:], self.AF.Copy)
            out.append(o)
        return out

    def linear_V(self, W, XT, work, psum):
        nc = self.nc
        out = []
        for t in range(NC):
            pt = psum.tile([P, D], self.F32, tag="lin", bufs=2)
            for c in range(NC):
                nc.tensor.matmul(pt[:], lhsT=XT[c][:, t * P:(t + 1) * P],
                                 rhs=W[c][:], start=(c == 0), stop=(c == NC - 1))
            v = work.tile([P, H * (DKH + 1)], self.BF16, tag=f"v{t}", bufs=2)
            src3 = pt[:].rearrange("p (h d) -> p h d", h=H)
            dst3 = v[:].rearrange("p (h d) -> p h d", h=H, d=DKH + 1)[:, :, 0:DKH]
            nc.vector.tensor_copy(dst3, src3)
            nc.vector.memset(
                v[:].rearrange("p (h d) -> p h d", h=H, d=DKH + 1)[:, :, DKH:DKH + 1],
                1.0)
            out.append(v)
        return out

    def _attention(self, QT, KT, Vt, work, psum, psum2, mask_mode, kbias,
                   tbias_tiles):
        nc, AF = self.nc, self.AF
        avT = [work.tile([P, D], self.BF16, tag=f"avT{c}", bufs=2,
                         name=f"avT{c}") for c in range(NC)]
        for h in range(H):
            ht, hp = h // 2, (h % 2) * DKH
            q_ap = QT[ht][hp:hp + DKH, :]
            avp = psum2.tile([DKH + 1, D], self.F32, tag="avp", bufs=1)
            for kc in range(NC):
                q0 = kc * P if mask_mode == "causal" else 0
                n = D - q0
                sp = psum.tile([P, D], self.F32, tag="sp", bufs=3)
                nc.tensor.matmul(sp[:, 0:n],
                                 lhsT=KT[ht][hp:hp + DKH, kc * P:(kc + 1) * P],
                                 rhs=q_ap[:, q0:D], start=True, stop=True)
                if mask_mode == "causal":
                    nc.vector.tensor_add(sp[:, 0:P], sp[:, 0:P],
                                         self.causal_diag[:])
                elif mask_mode == "full":
                    nc.vector.tensor_add(sp[:, 0:n], sp[:, 0:n],
                                         tbias_tiles[kc][:, q0:D])
                et = work.tile([P, D], self.BF16, tag="et", bufs=4)
                if mask_mode == "kbias":
                    nc.scalar.activation(et[:, 0:n], sp[:, 0:n], AF.Exp,
                                         bias=kbias[:, kc:kc + 1], scale=0.125)
                else:
                    nc.scalar.activation(et[:, 0:n], sp[:, 0:n], AF.Exp,
                                         scale=0.125)
                nc.tensor.matmul(avp[:, q0:D],
                                 lhsT=Vt[kc][:, h * (DKH + 1):(h + 1) * (DKH + 1)],
                                 rhs=et[:, 0:n], start=(kc == 0),
                                 stop=(kc == NC - 1))
            # avp rows 0..63 hold av^T (already the layout O-proj needs);
            # row 64 holds the softmax denominators. Broadcast 1/den to all
            # 64 partitions via a K=1 matmul, then one multiply finishes it.
            av_sb = work.tile([DKH, D], self.BF16, tag="av_sb", bufs=2)
            nc.scalar.activation(av_sb[:], avp[0:DKH, :], AF.Copy)
            den_sb = work.tile([1, D], self.F32, tag="den", bufs=2)
            nc.scalar.activation(den_sb[:], avp[DKH:DKH + 1, :], AF.Copy)
            rcp_f = work.tile([1, D], self.F32, tag="rcpf", bufs=2)
            nc.vector.reciprocal_approx_fast(out=rcp_f[:], in_=den_sb[:])
            rcp_b = work.tile([1, D], self.BF16, tag="rcpb", bufs=2)
            nc.vector.tensor_copy(rcp_b[:], rcp_f[:])
            r64 = psum2.tile([DKH, D], self.F32, tag="avTp", bufs=1)
            nc.tensor.matmul(r64[:], lhsT=self.ones64[:],
                             rhs=rcp_b[:], start=True, stop=True)
            nc.vector.tensor_tensor(out=avT[ht][hp:hp + DKH, :],
                                    in0=av_sb[:], in1=r64[:],
                                    op=self.OP.mult)
        return avT

    def oproj_resid(self, avT, Wo, X, psum, acc=None, res_scale=None,
                    acc_add=False):
        nc = self.nc
        for t in range(NC):
            pt = psum.tile([P, D], self.F32, tag="lin", bufs=2)
            for c in range(NC):
                nc.tensor.matmul(pt[:], lhsT=avT[c][:, t * P:(t + 1) * P],
                                 rhs=Wo[c][:], start=(c == 0), stop=(c == NC - 1))
            if acc is not None and not acc_add:
                nc.vector.scalar_tensor_tensor(
                    out=acc[t][:], in0=X[t][:], scalar=float(res_scale),
                    in1=pt[:], op0=self.OP.mult, op1=self.OP.add)
            elif acc is not None:
                nc.vector.tensor_add(acc[t][:], acc[t][:], pt[:])
            else:
                nc.vector.tensor_add(X[t][:], X[t][:], pt[:])

    def kv_proj(self, src_T, prefix, work, wpool, psum):
        WK = self.load_w(prefix + "kw", wpool, "wk")
        KT = self.linear_T(WK, src_T, work, psum, "kT")
        WV = self.load_w(prefix + "vw", wpool, "wv")
        Vt = self.linear_V(WV, src_T, work, psum)
        return KT, Vt

    def mha(self, YT_q, src_T, X, prefix, mask_mode, work, wpool, psum, psum2,
            kbias=None, tbias_tiles=None, acc=None, res_scale=None,
            acc_add=False, kv=None):
        """src_T: transposed source for K/V (y for self, mem for cross)."""
        if kv is None:
            kv = self.kv_proj(src_T, prefix, work, wpool, psum)
        KT, Vt = kv
        WQ = self.load_w(prefix + "qw", wpool, "wq")
        QT = self.linear_T(WQ, YT_q, work, psum, "qT")
        avT = self._attention(QT, KT, Vt, work, psum, psum2, mask_mode,
                              kbias, tbias_tiles)
        WO = self.load_w(prefix + "ow", wpool, "wo")
        self.oproj_resid(avT, WO, X, psum, acc=acc, res_scale=res_scale,
                         acc_add=acc_add)

    def ffn(self, X, prefix, work, wpool, psum):
        nc, AF = self.nc, self.AF
        Y = self.layernorm(X, work, self.BF16, unbiased=True, eps_in=0.0)
        YT = self.transpose_to(Y, work, psum)
        W1 = []
        for c in range(NC):
            w = wpool.tile([P, FF], self.BF16, tag=f"w1_{c}", bufs=1)
            nc.sync.dma_start(w[:], self.inputs[prefix + "w1"][c * P:(c + 1) * P, :])
            W1.append(w)
        H1 = []
        for oc in range(FF // P):
            pt = psum.tile([P, D], self.F32, tag="lin", bufs=2)
            for c in range(NC):
                nc.tensor.matmul(pt[:], lhsT=W1[c][:, oc * P:(oc + 1) * P],
                                 rhs=YT[c][:], start=(c == 0), stop=(c == NC - 1))
            h1 = work.tile([P, D], self.BF16, tag=f"h1_{oc}", bufs=1)
            nc.scalar.activation(h1[:], pt[:], AF.Relu)
            H1.append(h1)
        W2 = []
        for c in range(FF // P):
            w = wpool.tile([P, D], self.BF16, tag=f"w2_{c}", bufs=1)
            nc.sync.dma_start(w[:], self.inputs[prefix + "w2"][c * P:(c + 1) * P, :])
            W2.append(w)
        for t in range(NC):
            pt = psum.tile([P, D], self.F32, tag="lin", bufs=2)
            for c in range(FF // P):
                nc.tensor.matmul(pt[:], lhsT=H1[c][:, t * P:(t + 1) * P],
                                 rhs=W2[c][:], start=(c == 0),
                                 stop=(c == FF // P - 1))
            nc.vector.tensor_add(X[t][:], X[t][:], pt[:])

    def enc_layer(self, X, prefix, kbias, work, wpool, psum, psum2):
        Y = self.layernorm(X, work, self.BF16, unbiased=True, eps_in=0.0)
        YT = self.transpose_to(Y, work, psum)
        self.mha(YT, YT, X, prefix, "kbias", work, wpool, psum, psum2,
                 kbias=kbias)
        self.ffn(X, prefix, work, wpool, psum)

    def dec_layer(self, X, prefix, mem1T, mem2T, kb1, kb2, tbias, work,
                  wpool, psum, psum2, acc, xpool, xtag, kv1, next_prefix):
        Y = self.layernorm(X, work, self.BF16, unbiased=True, eps_in=0.0)
        YT = self.transpose_to(Y, work, psum)
        self.mha(YT, YT, X, prefix + "self_", self.cfg["tgt_mode"], work,
                 wpool, psum, psum2, tbias_tiles=tbias)
        kv2 = self.kv_proj(mem2T, prefix + "src2_", work, wpool, psum)
        Y1 = self.layernorm(X, work, self.BF16, unbiased=True, eps_in=0.0)
        YT1 = self.transpose_to(Y1, work, psum)
        self.mha(YT1, mem1T, X, prefix + "src1_", "kbias", work, wpool, psum,
                 psum2, kbias=kb1, acc=acc, res_scale=2.0, kv=kv1)
        Y2 = self.layernorm(X, work, self.BF16, unbiased=True, eps_in=0.0)
        YT2 = self.transpose_to(Y2, work, psum)
        self.mha(YT2, mem2T, X, prefix + "src2_", "kbias", work, wpool, psum,
                 psum2, kbias=kb2, acc=acc, acc_add=True, kv=kv2)
        Xn = self.layernorm(acc, work, self.F32, unbiased=False, eps_in=1e-5,
                            out_pool=xpool, out_tag=xtag)
        X[:] = Xn
        kv1_next = None
        if next_prefix is not None:
            # next layer's cross1 K/V: independent PE work that covers this
            # FFN's layernorm latency
            kv1_next = self.kv_proj(mem1T, next_prefix + "src1_", work,
                                    wpool, psum)
        self.ffn(X, prefix, work, wpool, psum)
        return kv1_next

    def generator(self, YT, lgpool, gwpool, gwork, psum):
        nc, AF = self.nc, self.AF
        out_dram = self.out
        vchunks = []
        off = 0
        while off < VOCAB:
            n = min(512, VOCAB - off)
            vchunks.append((off, n))
            off += n
        nvc = len(vchunks)
        VB = 4  # vchunks per block

        p2_count = [0]

        def pass2_chunk(prev, off, n):
            plg, pnlse, pt = prev
            ob = gwork.tile([P, 512], self.F32, tag="ob", bufs=4, name="ob")
            if p2_count[0] % 2 == 0:
                nc.vector.tensor_scalar_add(ob[:, 0:n], plg[:, off:off + n],
                                            pnlse[:])
            else:
                nc.scalar.activation(ob[:, 0:n], plg[:, off:off + n],
                                     AF.Identity, bias=pnlse[:])
            p2_count[0] += 1
            nc.sync.dma_start(out_dram[pt * P:(pt + 1) * P, off:off + n],
                              ob[:, 0:n])

        prev = None
        for t in range(NC):
            lg = lgpool.tile([P, VOCAB], self.BF16, tag="lg", bufs=2)
            se = gwork.tile([P, nvc], self.F32, tag="se", bufs=2)
            p2 = iter(vchunks if prev else ())
            for b0 in range(0, nvc, VB):
                block = vchunks[b0:b0 + VB]
                span0 = block[0][0]
                span_n = block[-1][0] + block[-1][1] - span0
                gw = []
                for c in range(NC):
                    w = gwpool.tile([P, 512 * VB], self.BF16, tag=f"gw{c}",
                                    bufs=2, name=f"gw{c}")
                    nc.sync.dma_start(
                        w[:, 0:span_n],
                        self.inputs["genw"][c * P:(c + 1) * P,
                                            span0:span0 + span_n])
                    gw.append(w)
                pts = [psum.tile([P, 512], self.F32, tag="glin", bufs=VB,
                                 name=f"glin{bi}") for bi in range(len(block))]
                for c in range(NC):
                    for bi, (off, n) in enumerate(block):
                        nc.tensor.matmul(pts[bi][:, 0:n],
                                         lhsT=YT[c][:, t * P:(t + 1) * P],
                                         rhs=gw[c][:, bi * 512:bi * 512 + n],
                                         start=(c == 0), stop=(c == NC - 1))
                for bi, (off, n) in enumerate(block):
                    trash = gwork.tile([P, 512], self.BF16, tag="gtrash",
                                       bufs=4)
                    nc.scalar.activation(trash[:, 0:n], pts[bi][:, 0:n],
                                         AF.Exp,
                                         accum_out=se[:, b0 + bi:b0 + bi + 1])
                    nc.vector.tensor_copy(lg[:, off:off + n], pts[bi][:, 0:n])
                    nxt = next(p2, None)
                    if nxt is not None:
                        pass2_chunk(prev, *nxt)
            for nxt in p2:
                pass2_chunk(prev, *nxt)
            ssum = gwork.tile([P, 1], self.F32, tag="ssum", bufs=2)
            nc.vector.reduce_sum(ssum[:], se[:], axis=self.AX.X)
            lse = gwork.tile([P, 1], self.F32, tag="lse", bufs=2)
            nc.scalar.activation(lse[:], ssum[:], AF.Ln)
            nlse = gwork.tile([P, 1], self.F32, tag="nlse", bufs=2)
            nc.vector.tensor_scalar_mul(nlse[:], lse[:], -1.0)
            prev = (lg, nlse, t)
        for off, n in vchunks:
            pass2_chunk(prev, off, n)


LDW_OPT = [False]


def _patch_toolchain():
    """Restrict ACT func-table sets to one covering set so the table is
    loaded once instead of being swapped between funcs (~1.5us per swap)."""
    import concourse.hw_specs as hw_specs
    import concourse.bacc as bacc_mod
    import concourse.bass_utils as bu
    from concourse import mybir
    AF = mybir.ActivationFunctionType
    needed = {AF.Exp, AF.Ln, AF.Copy, AF.Identity, AF.Relu}
    if getattr(bacc_mod, "_act_tables_patched", False) is False:
        orig = hw_specs.get_activation_tables

        def patched(arch):
            t = orig(arch)
            cover = [k for k, v in t.items() if needed <= v]
            if not cover:
                return t
            pick = cover[0]
            return {k: (v if k == pick else set()) for k, v in t.items()}

        bacc_mod.get_activation_tables = patched
        bacc_mod._act_tables_patched = True
    if LDW_OPT[0] and getattr(bu, "_ldw_patched", False) is False:
        orig_run = bu.run_command

        def run2(cmd, **kw):
            cmd = [c.replace("--enable-ldw-opt=false", "--enable-ldw-opt=true")
                   if isinstance(c, str) else c for c in cmd]
            return orig_run(cmd, **kw)

        bu.run_command = run2
        bu._ldw_patched = True


def build_model(cfg):
    import concourse.mybir as mybir
    import concourse.tile as tile
    from concourse import bacc
    from concourse.masks import make_identity

    _patch_toolchain()
    nc = bacc.Bacc("TRN2", target_bir_lowering=False, debug=False)
    with tile.TileContext(nc) as tc:
        b = Builder(nc, tc, cfg)
        F32, BF16, I32 = b.F32, b.BF16, b.I32
        for nm in ("src", "src2", "tgt"):
            b.dram_in(nm, [L, 1], I32)
        b.dram_in("pe", [L, D], F32)
        b.dram_in("kbias1", [P, NC], F32)
        b.dram_in("kbias2", [P, NC], F32)
        if cfg["tgt_mode"] == "causal":
            b.dram_in("causal_diag", [P, P], F32)
        elif cfg["tgt_mode"] == "full":
            b.dram_in("tbias", [L, L], F32)
        for e in (1, 2, 3):
            b.dram_in(f"emb{e}", [VOCAB, D], BF16)
        for enc in (1, 2):
            for j in range(cfg["n_enc"]):
                pfx = f"e{enc}{j}_"
                for w in ("qw", "kw", "vw", "ow"):
                    b.dram_in(pfx + w, [D, D], BF16)
                b.dram_in(pfx + "w1", [D, FF], BF16)
                b.dram_in(pfx + "w2", [FF, D], BF16)
        for j in range(cfg["n_dec"]):
            pfx = f"d{j}_"
            for att in ("self_", "src1_", "src2_"):
                for w in ("qw", "kw", "vw", "ow"):
                    b.dram_in(pfx + att + w, [D, D], BF16)
            b.dram_in(pfx + "w1", [D, FF], BF16)
            b.dram_in(pfx + "w2", [FF, D], BF16)
        if cfg["gen"]:
            b.dram_in("genw", [D, VOCAB], BF16)
        b.out = nc.dram_tensor("out", [L, VOCAB], mybir.dt.float32,
                               kind="ExternalOutput")
        dbg = cfg.get("dbg")
        if dbg:
            b.dbg = nc.dram_tensor("dbg", [L, D], mybir.dt.float32,
                                   kind="ExternalOutput")

        with tc.tile_pool(name="const", bufs=1) as const, \
             tc.tile_pool(name="xfer", bufs=1) as xfer:

            def dump_dbg(tiles):
                for t in range(NC):
                    ot = const.tile([P, D], F32, tag=f"dbgo{t}", bufs=1)
                    nc.scalar.activation(ot[:], tiles[t][:], b.AF.Copy)
                    nc.sync.dma_start(b.dbg[t * P:(t + 1) * P, :], ot[:])

            b.const_pool = const
            b.idn = const.tile([P, P], BF16, tag="idn", bufs=1)
            make_identity(nc, b.idn[:])
            b.ones64 = const.tile([1, DKH], BF16, tag="ones64", bufs=1)
            nc.vector.memset(b.ones64[:], 1.0)
            kb1 = const.tile([P, NC], F32, tag="kb1", bufs=1)
            nc.sync.dma_start(kb1[:], b.inputs["kbias1"][:])
            kb2 = const.tile([P, NC], F32, tag="kb2", bufs=1)
            nc.sync.dma_start(kb2[:], b.inputs["kbias2"][:])
            tbias = None
            if cfg["tgt_mode"] == "causal":
                cd = const.tile([P, P], F32, tag="cd", bufs=1)
                nc.sync.dma_start(cd[:], b.inputs["causal_diag"][:])
                b.causal_diag = cd
            elif cfg["tgt_mode"] == "full":
                tbias = []
                for kc in range(NC):
                    tb = const.tile([P, L], F32, tag=f"tb{kc}", bufs=1)
                    nc.sync.dma_start(tb[:], b.inputs["tbias"][kc * P:(kc + 1) * P, :])
                    tbias.append(tb)

            with tc.tile_pool(name="encdec", bufs=2) as work, \
                 tc.tile_pool(name="wpool", bufs=1) as wpool, \
                 tc.tile_pool(name="xpool", bufs=1) as xpool, \
                 tc.tile_pool(name="pepool", bufs=1) as pepool, \
                 tc.tile_pool(name="psum", bufs=1, space="PSUM") as psum, \
                 tc.tile_pool(name="psum2", bufs=1, space="PSUM") as psum2:
                pe_tiles = []
                for t in range(NC):
                    pt_ = pepool.tile([P, D], F32, tag=f"pe{t}", bufs=1)
                    nc.sync.dma_start(pt_[:], b.inputs["pe"][t * P:(t + 1) * P, :])
                    pe_tiles.append(pt_)

                X1 = b.embed(xpool, "x1_", b.inputs["src"], b.inputs["emb1"],
                             pe_tiles, work)
                X2 = b.embed(xpool, "x2_", b.inputs["src2"], b.inputs["emb2"],
                             pe_tiles, work)
                if dbg == "emb1":
                    dump_dbg(X1)
                if dbg == "emb2":
                    dump_dbg(X2)
                # interleave the two (independent) encoders so one's matmuls
                # cover the other's layernorm latency
                for j in range(cfg["n_enc"]):
                    b.enc_layer(X1, f"e1{j}_", kb1, work, wpool, psum, psum2)
                    b.enc_layer(X2, f"e2{j}_", kb2, work, wpool, psum, psum2)
                if dbg == "enc1":
                    dump_dbg(b.layernorm(X1, work, F32, unbiased=True,
                                         eps_in=0.0, out_tag="yfd"))
                if dbg == "enc2":
                    dump_dbg(b.layernorm(X2, work, F32, unbiased=True,
                                         eps_in=0.0, out_tag="yfd"))
                Yf1 = b.layernorm(X1, work, BF16, unbiased=True, eps_in=0.0)
                mem1T = b.transpose_to(Yf1, work, psum, out_tag="m1_",
                                       out_pool=xfer)
                Yf2 = b.layernorm(X2, work, BF16, unbiased=True, eps_in=0.0)
                mem2T = b.transpose_to(Yf2, work, psum, out_tag="m2_",
                                       out_pool=xfer)

                Xd = b.embed(xpool, "x1_", b.inputs["tgt"], b.inputs["emb3"],
                             pe_tiles, work)
                if dbg == "emb3":
                    dump_dbg(Xd)
                acc = [xpool.tile([P, D], F32, tag=f"acc{t}", bufs=1,
                                  name=f"acc{t}") for t in range(NC)]
                kv1 = b.kv_proj(mem1T, "d0_src1_", work, wpool, psum)
                for j in range(cfg["n_dec"]):
                    nxt = f"d{j + 1}_" if j + 1 < cfg["n_dec"] else None
                    kv1 = b.dec_layer(Xd, f"d{j}_", mem1T, mem2T, kb1, kb2,
                                      tbias, work, wpool, psum, psum2, acc,
                                      xpool, "x1_", kv1, nxt)
                if dbg == "dec":
                    Yd32 = b.layernorm(Xd, work, F32, unbiased=True,
                                       eps_in=0.0, out_tag="ydd")
                    dump_dbg(Yd32)
                Yg = b.layernorm(Xd, work, BF16, unbiased=True, eps_in=0.0)
                YgT = b.transpose_to(Yg, work, psum, out_tag="ygT",
                                     out_pool=xfer)

            if cfg["gen"]:
                with tc.tile_pool(name="lg", bufs=1) as lgpool, \
                     tc.tile_pool(name="gw", bufs=1) as gwpool, \
                     tc.tile_pool(name="gwork", bufs=1) as gwork, \
                     tc.tile_pool(name="psumg", bufs=1, space="PSUM") as psumg:
                    b.generator(YgT, lgpool, gwpool, gwork, psumg)
    nc.compile()
    return nc


# -------------------------------------------------------------------- host --
def _pos_encoding(max_len, d):
    pos = np.arange(max_len, dtype=np.float32)[:, None]
    div = np.exp(np.arange(0, d, 2, dtype=np.float32) * (-np.log(10000.0) / d))
    pe = np.zeros((max_len, d), np.float32)
    pe[:, 0::2] = np.sin(pos * div)
    pe[:, 1::2] = np.cos(pos * div)
    return pe


def _prep_weights(params):
    import ml_dtypes
    bf16 = ml_dtypes.bfloat16
    out = {}
    nonzero_bias = False
    ln_affine = False

    def lin(dst, p):
        nonlocal nonzero_bias
        w = np.asarray(p["w"], np.float32)
        out[dst] = np.ascontiguousarray(w.T).astype(bf16)
        if np.any(np.asarray(p["b"]) != 0):
            nonzero_bias = True

    def ln_check(p):
        nonlocal ln_affine
        if np.any(np.asarray(p["a"]) != 1) or np.any(np.asarray(p["b"]) != 0):
            ln_affine = True

    for e in (1, 2, 3):
        emb = np.asarray(params[f"emb{e}"], np.float32) * SQD
        out[f"emb{e}"] = emb.astype(bf16)
    n_enc = len(params["enc1"]["layers"])
    for enc in (1, 2):
        pp = params[f"enc{enc}"]
        for j, lp in enumerate(pp["layers"]):
            pfx = f"e{enc}{j}_"
            for k, nm in (("q", "qw"), ("k", "kw"), ("v", "vw"), ("o", "ow")):
                lin(pfx + nm, lp["attn"][k])
            lin(pfx + "w1", lp["ff"]["w1"])
            lin(pfx + "w2", lp["ff"]["w2"])
            ln_check(lp["n1"]); ln_check(lp["n2"])
        ln_check(pp["norm"])
    n_dec = len(params["dec"]["layers"])
    for j, lp in enumerate(params["dec"]["layers"]):
        pfx = f"d{j}_"
        for att, key in (("self_", "self"), ("src1_", "src1"), ("src2_", "src2")):
            for k, nm in (("q", "qw"), ("k", "kw"), ("v", "vw"), ("o", "ow")):
                lin(pfx + att + nm, lp[key][k])
        lin(pfx + "w1", lp["ff"]["w1"])
        lin(pfx + "w2", lp["ff"]["w2"])
        for n in lp["n"]:
            ln_check(n)
        ln_check(lp["lay"])
    ln_check(params["dec"]["norm"])
    lin("genw", params["gen"])
    return out, n_enc, n_dec, nonzero_bias, ln_affine


def kernel(src, src2, tgt, src_mask, src2_mask, tgt_mask, params):
    src = np.asarray(src); src2 = np.asarray(src2); tgt = np.asarray(tgt)
    src_mask = np.asarray(src_mask); src2_mask = np.asarray(src2_mask)
    tgt_mask = np.asarray(tgt_mask)
    B = src.shape[0]

    weights, n_enc, n_dec, nonzero_bias, ln_affine = _prep_weights(params)
    if nonzero_bias or ln_affine:
        raise NotImplementedError(
            "nonzero linear bias / affine LN not supported by this kernel")

    tril = np.tril(np.ones((L, L), np.int32))
    causal = all(np.array_equal(np.asarray(tgt_mask[i]).astype(np.int32), tril)
                 for i in range(B))
    tgt_any_mask = bool(np.any(np.asarray(tgt_mask) == 0))
    tgt_mode = "causal" if causal else ("full" if tgt_any_mask else "none")

    cfg_key = (n_enc, n_dec, tgt_mode)
    if cfg_key not in _COMPILED:
        cfg = {"n_enc": n_enc, "n_dec": n_dec, "tgt_mode": tgt_mode,
               "gen": True, "dbg": None}
        _COMPILED[cfg_key] = build_model(cfg)
    nc = _COMPILED[cfg_key]

    pe = _pos_encoding(L, D)
    q = np.arange(P, dtype=np.float32)
    causal_diag = np.where(q[:, None] > q[None, :], -8e9, 0.0).astype(np.float32)

    in_maps = []
    for i in range(B):
        m = dict(weights)
        m["src"] = np.ascontiguousarray(src[i].reshape(L, 1)).astype(np.int32)
        m["src2"] = np.ascontiguousarray(src2[i].reshape(L, 1)).astype(np.int32)
        m["tgt"] = np.ascontiguousarray(tgt[i].reshape(L, 1)).astype(np.int32)
        m["pe"] = pe
        m["kbias1"] = np.ascontiguousarray(
            ((src_mask[i, 0].astype(np.float32) - 1) * 1e9).reshape(NC, P).T)
        m["kbias2"] = np.ascontiguousarray(
            ((src2_mask[i, 0].astype(np.float32) - 1) * 1e9).reshape(NC, P).T)
        if tgt_mode == "causal":
            m["causal_diag"] = causal_diag
        elif tgt_mode == "full":
            m["tbias"] = np.ascontiguousarray(
                (tgt_mask[i].astype(np.float32).T - 1) * 8e9)
        in_maps.append(m)

    res = _run_spmd(nc, in_maps, trace=TRACE[0])
    LAST_EXEC_NS[0] = res.exec_time_ns
    LAST_RES[0] = res
    return np.stack([res.results[i]["out"] for i in range(B)]).astype(np.float32)
